# revision 11
# baseline (speedup 1.0000x reference)
"""Trainium2 Bass kernel for the CAER ragged-sequence problem.

Computes, for full inputs (B=64, S=2048, D=300, V=50000, SA=8):
  text_len = count_nonzero(text_indices, axis=1)
  text     = embed_table[text_indices]                    # [B,S,D]
  u        = text @ fc_w.T + fc_b                         # [B,S,1]
  mean_u   = masked mean of u over the valid prefix
  weight   = (u >= mean_u) & mask
  output   = text * weight * u                            # [B,S,D]
  orig_target_embed = embed_table[target_indices]         # [B,SA,D]

Sharding: pure data parallel — batch dim split across 8 NeuronCores,
embed table + fc weights replicated.

Per-core kernel (8 batch rows), per row:
  - 16 indirect-DMA gathers (one offset per dest partition) fetch the
    2048 token embeddings into SBUF [128 part x 16 tok x 300 dim];
    padding tokens (remapped to index V) are skipped via bounds-check.
  - DVE dot with fc_w (+ ACT accum-reduce), PE ones-matmul for the
    cross-partition masked sums, threshold via u*L >= S (no division),
  - scale rows in place, DMA the [128, 4800] block back out.
"""

import dataclasses
from contextlib import ExitStack

import numpy as np

import concourse.bass as bass
import concourse.bacc as bacc
import concourse.mybir as mybir
import concourse.tile as tile

B, S, D, V, SA = 64, 2048, 300, 50000, 8
NCORES = 8
RPC = B // NCORES          # batch rows per core
P = 128                    # SBUF partitions
C = S // P                 # tokens per partition per row (16)
DT = mybir.dt.float32
F32 = mybir.AluOpType

# Production config (mirrored by make_in_maps's index remap).
CFG = dict(
    gather=True,        # issue the embedding gathers
    compute=True,       # run the u/mean/threshold/scale math
    store=True,         # DMA results to DRAM
    skip_pad=True,      # remap padding idx 0 -> V and bounds-skip the fetch
    act_reduce=True,    # u chunk reduction on ScalarE (else DVE tensor_reduce)
    act_final=8,        # how many of the C final-scale chunks go to ScalarE
    row_bufs=3,
    queues=1,           # SWDGE queues to spread indirect gathers over
    prs=(P,) * (B // NCORES),  # baked per-row-slot gather partition counts
)


def _with_ap(ap, ap_list):
    """Return a copy of `ap` with a hand-built [step, count] pattern."""
    return dataclasses.replace(ap, ap=ap_list)


def _body(nc, tc, idx, tgt, table, w, b, out_text, out_tgt, cfg=CFG):
    pad_idx = V if cfg["skip_pad"] else 0
    with ExitStack() as ctx:
        const = ctx.enter_context(tc.tile_pool(name="const", bufs=1))
        idxp = ctx.enter_context(tc.tile_pool(name="idxp", bufs=2))
        rowp = ctx.enter_context(tc.tile_pool(name="rowp", bufs=cfg["row_bufs"]))
        scrp = ctx.enter_context(tc.tile_pool(name="scrp", bufs=2))
        smap = ctx.enter_context(tc.tile_pool(name="small", bufs=4))
        psp = ctx.enter_context(tc.tile_pool(name="psum", bufs=4, space="PSUM"))

        # fc_w broadcast to all partitions; ones vectors for PE reductions.
        w_bc = const.tile([P, D], DT)
        nc.sync.dma_start(w_bc[0:1, :], w[:])
        nc.gpsimd.partition_broadcast(w_bc[:], w_bc[0:1, :])
        b_bc = const.tile([P, 1], DT)
        nc.sync.dma_start(b_bc[0:1, :], b[:])
        nc.gpsimd.partition_broadcast(b_bc[:], b_bc[0:1, :])
        ones_col = const.tile([P, 1], DT)
        nc.vector.memset(ones_col[:], 1.0)
        ones_row = const.tile([1, P], DT)
        nc.vector.memset(ones_row[:], 1.0)
        if any(p < P for p in cfg["prs"]):
            zero_tile = const.tile([P, C * D], DT)
            nc.vector.memset(zero_tile[:], 0.0)

        if cfg["skip_pad"]:
            # Bounds-skipped slots keep stale SBUF bytes; zero the row pool
            # slots once so the first rows never see non-finite garbage.
            for _ in range(cfg["row_bufs"]):
                warm = rowp.tile([P, C * D], DT, tag="rows")
                nc.vector.memset(warm[:], 0.0)

        # orig_target_embed: one small gather for this core's 64 targets.
        if cfg["gather"] and cfg["store"]:
            tgt_sb = smap.tile([RPC * SA, 1], mybir.dt.int32, tag="tgtidx")
            nc.sync.dma_start(tgt_sb[:], tgt[:])
            tgt_rows = smap.tile([RPC * SA, D], DT, tag="tgtrows")
            nc.gpsimd.indirect_dma_start(
                tgt_rows[:],
                None,
                table[:],
                bass.IndirectOffsetOnAxis(ap=tgt_sb[:], axis=0),
            )
            nc.sync.dma_start(out_tgt[:], tgt_rows[:])

        for r in range(RPC):
            idx_sb = idxp.tile([P, C], mybir.dt.int32)
            nc.sync.dma_start(idx_sb[:], idx[r])

            # Gather 2048 rows of 300 f32; token t=(p*C+c) -> partition p,
            # chunk c. HW indirect DMA consumes ONE offset per dest
            # partition (src walks contiguously within a partition), so
            # issue C calls of [128 offsets] -> [128, D] each.
            rows = rowp.tile([P, C * D], DT, tag="rows")
            pr = cfg["prs"][r]
            if cfg["gather"]:
                for c in range(C):
                    inst = nc.gpsimd.indirect_dma_start(
                        rows[0:pr, c * D : (c + 1) * D],
                        None,
                        table[:],
                        bass.IndirectOffsetOnAxis(ap=idx_sb[0:pr, c : c + 1], axis=0),
                        bounds_check=V - 1 if cfg["skip_pad"] else None,
                        oob_is_err=False,
                    )
                    q = c % cfg["queues"]
                    if q:
                        inst.ins.queue = f"qPoolDynamic{q}"
            if not cfg["compute"]:
                if cfg["store"]:
                    if not cfg["gather"]:
                        nc.vector.memset(rows[:], 0.0)
                    nc.sync.dma_start(out_text[r], rows[:])
                continue

            rows_ap = rows[0:pr, :]
            rows3 = rows_ap.rearrange("p (c d) -> p c d", c=C)

            stats = smap.tile([P, 2 * C], DT, tag="stats")
            if pr < P:
                nc.vector.memset(stats[:], 0.0)
            mask = stats[0:pr, C : 2 * C]
            mu = stats[0:pr, 0:C]
            # mask = (idx != pad) as f32
            nc.vector.tensor_scalar(
                out=mask, in0=idx_sb[0:pr, :], scalar1=pad_idx, scalar2=None,
                op0=F32.not_equal,
            )

            # u[p,c] = sum_d rows[p,c,d] * w[d]
            scr = scrp.tile([P, C * D], DT)
            w3 = _with_ap(w_bc[0:pr, :], [w_bc[0:pr, :].ap[0], [0, C], [1, D]])
            nc.vector.tensor_tensor(
                out=scr[0:pr, :].rearrange("p (c d) -> p c d", c=C),
                in0=rows3, in1=w3, op=F32.mult,
            )
            u = smap.tile([P, C], DT, tag="u")
            if cfg["act_reduce"]:
                dump = scrp.tile([P, D], DT, tag="actdump")
                for c in range(C):
                    nc.scalar.activation(
                        dump[0:pr, :], scr[0:pr, c * D : (c + 1) * D],
                        mybir.ActivationFunctionType.Identity,
                        accum_out=u[0:pr, c : c + 1],
                    )
            else:
                nc.vector.tensor_reduce(
                    out=u[0:pr, :],
                    in_=scr[0:pr, :].rearrange("p (c d) -> p c d", c=C),
                    axis=mybir.AxisListType.X,
                    op=F32.add,
                )
            nc.vector.tensor_tensor(out=mu, in0=u[0:pr, :], in1=mask, op=F32.mult)

            # Cross-partition sums of [mu | mask] via ones-matmul: [1, 2C].
            ps = psp.tile([1, 2 * C], DT, tag="ps")
            nc.tensor.matmul(
                out=ps[:], lhsT=ones_col[:], rhs=stats[:], start=True, stop=True
            )
            sl = smap.tile([1, 2], DT, tag="sl")  # [S_sum, L_len]
            nc.vector.tensor_reduce(
                out=sl[:],
                in_=ps[:].rearrange("p (a c) -> p a c", a=2),
                axis=mybir.AxisListType.X,
                op=F32.add,
            )
            # Broadcast [S, L] to all partitions via matmul with ones.
            psm = psp.tile([P, 2], DT, tag="psm")
            nc.tensor.matmul(
                out=psm[:], lhsT=ones_row[:], rhs=sl[:], start=True, stop=True
            )

            # weight = (u*L >= S) & mask  (equivalent to u >= S/L, L>0)
            uL = smap.tile([P, C], DT, tag="uL")
            nc.vector.tensor_scalar(
                out=uL[0:pr, :], in0=u[0:pr, :], scalar1=psm[0:pr, 1:2],
                scalar2=None, op0=F32.mult,
            )
            w1 = smap.tile([P, C], DT, tag="w1")
            nc.vector.scalar_tensor_tensor(
                out=w1[0:pr, :], in0=uL[0:pr, :], scalar=psm[0:pr, 0:1], in1=mask,
                op0=F32.is_ge, op1=F32.mult,
            )
            # scale = (u + b) * weight
            scale = smap.tile([P, C], DT, tag="scale")
            nc.vector.scalar_tensor_tensor(
                out=scale[0:pr, :], in0=u[0:pr, :], scalar=b_bc[0:pr, 0:1],
                in1=w1[0:pr, :], op0=F32.add, op1=F32.mult,
            )

            # rows[0:pr] *= scale (broadcast along D), in place; ScalarE takes
            # act_final chunks (per-partition scale), DVE the rest.
            nact = cfg["act_final"]
            for c in range(nact):
                nc.scalar.activation(
                    rows[0:pr, c * D : (c + 1) * D],
                    rows[0:pr, c * D : (c + 1) * D],
                    mybir.ActivationFunctionType.Copy,
                    scale=scale[0:pr, c : c + 1],
                )
            if nact < C:
                base = rows[0:pr, nact * D :]
                sub3 = _with_ap(base, [base.ap[0], [D, C - nact], [1, D]])
                base_s = scale[0:pr, nact:]
                scale3 = _with_ap(base_s, [base_s.ap[0], [1, C - nact], [0, D]])
                nc.vector.tensor_tensor(
                    out=sub3, in0=sub3, in1=scale3, op=F32.mult
                )
            if cfg["store"]:
                nc.sync.dma_start(out_text[r][0:pr, :], rows[0:pr, :])
                if pr < P:
                    nc.sync.dma_start(
                        out_text[r][pr:P, :], zero_tile[0 : P - pr, :]
                    )


def build(cfg=CFG):
    nc = bacc.Bacc(
        "TRN2", target_bir_lowering=False, debug=False, enable_asserts=False,
        num_swdge_queues=cfg.get("queues", 1),
    )
    idx = nc.dram_tensor("idx", [RPC, P, C], mybir.dt.int32, kind="ExternalInput").ap()
    tgt = nc.dram_tensor("tgt", [RPC * SA, 1], mybir.dt.int32, kind="ExternalInput").ap()
    table = nc.dram_tensor("table", [V, D], DT, kind="ExternalInput").ap()
    w = nc.dram_tensor("w", [1, D], DT, kind="ExternalInput").ap()
    b = nc.dram_tensor("b", [1, 1], DT, kind="ExternalInput").ap()
    out_text = nc.dram_tensor("out_text", [RPC, P, C * D], DT, kind="ExternalOutput").ap()
    out_tgt = nc.dram_tensor("out_tgt", [RPC * SA, D], DT, kind="ExternalOutput").ap()
    with tile.TileContext(nc) as tc:
        _body(nc, tc, idx, tgt, table, w, b, out_text, out_tgt, cfg)
    nc.compile()
    return nc


_NC = None
_NC_PRS = None


def _get_nc(prs):
    global _NC, _NC_PRS
    if _NC is None or _NC_PRS != prs:
        _NC = build(dict(CFG, prs=prs))
        _NC_PRS = prs
    return _NC


def plan_rows(text_indices):
    """Row planning. Baked partial-partition gathers measured SLOWER on HW
    (SWDGE cost is per-call fixed; partial-partition DMAs lose SBUF-port
    parallelism), so run every row slot at the full 128 partitions."""
    return None, (P,) * RPC


def make_in_maps(text_indices, target_indices, embed_table, fc_w, fc_b, cfg=CFG,
                 perms=None):
    ti = np.asarray(text_indices).astype(np.int32)
    if cfg["skip_pad"]:
        ti = np.where(ti == 0, np.int32(V), ti)
    ti = np.ascontiguousarray(ti).reshape(NCORES, RPC, P, C)
    if perms is not None:
        ti = np.ascontiguousarray(
            np.take_along_axis(ti, perms[:, :, None, None], axis=1)
        )
    tg = np.ascontiguousarray(np.asarray(target_indices).astype(np.int32)).reshape(
        NCORES, RPC * SA, 1
    )
    table = np.ascontiguousarray(np.asarray(embed_table).astype(np.float32))
    w = np.ascontiguousarray(np.asarray(fc_w).astype(np.float32)).reshape(1, D)
    b = np.ascontiguousarray(np.asarray(fc_b).astype(np.float32)).reshape(1, 1)
    return [
        {"idx": ti[k], "tgt": tg[k], "table": table, "w": w, "b": b}
        for k in range(NCORES)
    ]


def kernel(
    text_indices, aspect_indices, target_indices, embed_table, fc_w, fc_b
):
    from concourse import bass_utils

    perms, prs = plan_rows(text_indices)
    nc = _get_nc(prs)
    in_maps = make_in_maps(
        text_indices, target_indices, embed_table, fc_w, fc_b, perms=perms
    )
    res = bass_utils.run_bass_kernel_spmd(nc, in_maps, core_ids=list(range(NCORES)))
    outs = res.results
    out_text = np.concatenate(
        [outs[k]["out_text"].reshape(RPC, S, D) for k in range(NCORES)], axis=0
    )
    out_tgt = np.concatenate(
        [outs[k]["out_tgt"].reshape(RPC, SA, D) for k in range(NCORES)], axis=0
    )
    return out_text, out_tgt
